# revision 11
# baseline (speedup 1.0000x reference)
"""Trainium2 Bass kernel for nn_AttentionBlock (GroupNorm + MHSA + proj + residual).

Data-parallel over batch: B=8 batch elements -> 8 NeuronCores, one each.
Per core (fp32 activations/psum, bf16 matmul operands):
  x_b [C=512, N=1024]
  group_norm (32 groups of 16 channels; groups live within one 128-partition
  chunk, so stats/normalization pipeline with the x DMA per chunk)
  qkv = W_qkv @ x_norm + b  (q,k in [c,n] layout; v computed directly transposed [m,c])
  per head h (8 heads, hd=64): S^T = k_h^T q_h  [m,n]; P = exp(S^T/8);
  out_h = v_h @ P / colsum (colsum via a ones-column appended to v^T -> M=65 matmul)
  y = x + W_proj @ out + b_proj

The attention loop is paced by the Scalar engine (64 exp ACTIVATEs of
[128,1024] ~= 73us is the hard floor). Everything else hides under it:
QK of iteration i+1 is emitted before AV of iteration i; the QKV projection
groups are split into 2-matmul halves and interleaved just-in-time so each
iteration's PE work fits inside one exp; weight DMAs for the later-needed
columns are gated behind the x transfer so GroupNorm starts ASAP; per-head
normalization chains are emitted in deferred pieces so they never convoy
ahead of latency-critical bias-adds on the Vector FIFO; the final head-pair
normalizes straight out of PSUM to shorten the tail.
"""

import numpy as np

C = 512
N = 1024  # H*W
NH = 8
HD = 64
NG = 32
EPS = 1e-5
NCORES = 8

_CACHE = {}

# wqkvT host column order: [q0 | k0 | v(512) | k1 k2 k3 | q1 q2 q3]
def _qcol(j):
    return 0 if j == 0 else 1152 + (j - 1) * 128

def _kcol(j):
    return 128 if j == 0 else 768 + (j - 1) * 128

VCOL = 256  # v section: cols 256:768


def _build_program():
    import concourse.bass as bass  # noqa: F401
    import concourse.mybir as mybir
    import concourse.tile as tile
    from concourse import bacc

    f32 = mybir.dt.float32
    bf16 = mybir.dt.bfloat16
    Act = mybir.ActivationFunctionType
    from concourse.alu_op_type import AluOpType as Op

    nc = bacc.Bacc("TRN2", target_bir_lowering=False, debug=False, num_devices=NCORES)

    x_d = nc.dram_tensor("x", [128, 4, N], bf16, kind="ExternalInput")
    wqkvT_d = nc.dram_tensor("wqkvT", [128, 4, 3 * C], bf16, kind="ExternalInput")
    wprojT_d = nc.dram_tensor("wprojT", [128, 4, C], bf16, kind="ExternalInput")
    cst_d = nc.dram_tensor("cst", [128, 20], f32, kind="ExternalInput")
    bvb_d = nc.dram_tensor("bvb", [128, 512], bf16, kind="ExternalInput")
    gsel_d = nc.dram_tensor("gsel", [128, 8], bf16, kind="ExternalInput")
    bsel_d = nc.dram_tensor("bsel", [8, 128], bf16, kind="ExternalInput")
    y_d = nc.dram_tensor("y", [C, N], mybir.dt.bfloat16, kind="ExternalOutput")
    dbg_d = nc.dram_tensor("dbg", [C, N], mybir.dt.bfloat16, kind="ExternalOutput")

    with tile.TileContext(nc) as tc:
        with tc.tile_pool(name="mem", bufs=1) as mem:
            # ---- persistent tiles ----
            xf_t = mem.tile([128, 4, N], bf16, tag="xf", name="xf")
            x_t = [xf_t[:, k, :] for k in range(4)]
            xn_t = [mem.tile([128, N], bf16, tag=f"xn{k}", name=f"xn{k}") for k in range(4)]
            wqf_t = mem.tile([128, 4, 3 * C], bf16, tag="wqf", name="wqf")
            wq_t = [wqf_t[:, k, :] for k in range(4)]
            wpf_t = mem.tile([128, 4, C], bf16, tag="wpf", name="wpf")
            wp_t = [wpf_t[:, k, :] for k in range(4)]
            cst_t = mem.tile([128, 20], f32, tag="cst", name="cst")
            gam_t = cst_t[:, 0:4]
            bet_t = cst_t[:, 4:8]
            bqT_t = cst_t[:, 8:16]
            bpT_t = cst_t[:, 16:20]
            bvb_t = mem.tile([128, 512], bf16, tag="bvb", name="bvb")
            gsel_t = mem.tile([128, 8], bf16, tag="gsel", name="gsel")
            bsel_t = mem.tile([8, 128], bf16, tag="bsel", name="bsel")
            # query halves per (nt, head-pair); key full tiles per head-pair
            qh_t = [[mem.tile([128, 512], bf16, tag=f"q{j}n{nt}", name=f"q{j}n{nt}")
                     for j in range(4)] for nt in range(2)]
            kf_t = [mem.tile([128, N], bf16, tag=f"kf{j}", name=f"kf{j}") for j in range(4)]
            # v^T with a ones column appended per head: [128, mc, 8 heads, 65]
            vpf_t = mem.tile([128, 8, NH, HD + 1], bf16, tag="vpf", name="vpf")
            out_t = [mem.tile([128, N], bf16, tag=f"out{hp}", name=f"out{hp}") for hp in range(4)]
            y_t = [mem.tile([128, N], bf16, tag=f"y{k}", name=f"y{k}") for k in range(4)]

            # ---- input DMAs: x first (it gates GroupNorm); the first-needed
            # weight columns (q0,k0,v) race behind it; the rest of the weights
            # are gated on the x transfer (WAW dep on the memset below) so
            # they don't steal HBM bandwidth from x ----
            nc.sync.dma_start(out=xf_t[:, 0:1, :], in_=x_d[:, 0:1, :])
            nc.scalar.dma_start(out=xf_t[:, 2:3, :], in_=x_d[:, 2:3, :])
            nc.sync.dma_start(out=xf_t[:, 1:2, :], in_=x_d[:, 1:2, :])
            nc.scalar.dma_start(out=xf_t[:, 3:4, :], in_=x_d[:, 3:4, :])
            nc.sync.dma_start(out=wqf_t[:, :, 0:768], in_=wqkvT_d[:, :, 0:768])
            nc.gpsimd.dma_start(out=cst_t, in_=cst_d[:])
            nc.gpsimd.dma_start(out=gsel_t, in_=gsel_d[:])
            nc.gpsimd.dma_start(out=bsel_t, in_=bsel_d[:])
            nc.gpsimd.dma_start(out=bvb_t, in_=bvb_d[:])
            nc.vector.memset(vpf_t[:, :, :, HD:HD + 1], 1.0)

            # ---- group norm (independent per 128-channel chunk) ----
            with (
                tc.tile_pool(name="gn", bufs=1) as gn,
                tc.tile_pool(name="pgn", bufs=2, space="PSUM") as pgn,
            ):
                eps_t = gn.tile([8, 1], f32, tag="eps", name="eps")
                nc.vector.memset(eps_t, EPS)
                for k in range(4):
                    st = gn.tile([128, 2, 6], f32, tag=f"st{k}", name=f"st{k}")
                    for j in range(2):
                        nc.vector.bn_stats(out=st[:, j, :], in_=x_t[k][:, j * 512:(j + 1) * 512])
                    if k == 3:
                        # x fully landed once chunk-3 stats ran: release the
                        # gated weight DMAs (WAW dep through these memsets)
                        nc.vector.memset(wqf_t[:, 0:1, 768:772], 0.0)
                        nc.vector.memset(wpf_t[:, 0:1, 0:4], 0.0)
                        nc.sync.dma_start(out=wqf_t[:, :, 768:1536], in_=wqkvT_d[:, :, 768:1536])
                        nc.sync.dma_start(out=wpf_t, in_=wprojT_d[:])
                    mv = gn.tile([128, 2], f32, tag=f"mv{k}", name=f"mv{k}")
                    nc.vector.bn_aggr(out=mv, in_=st)
                    s2 = gn.tile([128, 2], bf16, tag=f"s2{k}", name=f"s2{k}")
                    nc.vector.tensor_copy(out=s2[:, 0:1], in_=mv[:, 0:1])
                    nc.vector.tensor_scalar(out=s2[:, 1:2], in0=mv[:, 0:1], scalar1=mv[:, 0:1],
                                            scalar2=mv[:, 1:2], op0=Op.mult, op1=Op.add)
                    mvp = pgn.tile([8, 2], f32, tag="mvp", name="mvp")
                    nc.tensor.matmul(mvp, gsel_t, s2, start=True, stop=True)
                    gnm = gn.tile([8, 2], f32, tag=f"gnm{k}", name=f"gnm{k}")
                    nc.vector.tensor_copy(out=gnm, in_=mvp)
                    gn2b = gn.tile([8, 2], bf16, tag=f"gn2b{k}", name=f"gn2b{k}")
                    nc.vector.tensor_copy(out=gn2b[:, 0:1], in_=gnm[:, 0:1])
                    var8 = gn.tile([8, 1], f32, tag=f"var{k}", name=f"var{k}")
                    nc.vector.tensor_tensor(out=var8, in0=gnm[:, 0:1], in1=gnm[:, 0:1], op=Op.mult)
                    nc.vector.tensor_tensor(out=var8, in0=gnm[:, 1:2], in1=var8, op=Op.subtract)
                    nc.scalar.activation(out=var8, in_=var8, func=Act.Sqrt, bias=eps_t, scale=1.0)
                    rst8 = gn.tile([8, 1], f32, tag=f"rst{k}", name=f"rst{k}")
                    nc.vector.reciprocal_approx_fast(out=rst8, in_=var8)
                    nc.vector.tensor_copy(out=gn2b[:, 1:2], in_=rst8)
                    bcp = pgn.tile([128, 2], f32, tag="bcp", name="bcp")
                    nc.tensor.matmul(bcp, bsel_t, gn2b, start=True, stop=True)
                    sc = gn.tile([128, 1], f32, tag=f"sc{k}", name=f"sc{k}")
                    tcv = gn.tile([128, 1], f32, tag=f"tc{k}", name=f"tc{k}")
                    nc.vector.tensor_tensor(out=sc, in0=bcp[:, 1:2], in1=gam_t[:, k:k + 1], op=Op.mult)
                    nc.vector.tensor_tensor(out=tcv, in0=bcp[:, 0:1], in1=sc, op=Op.mult)
                    nc.vector.tensor_tensor(out=tcv, in0=bet_t[:, k:k + 1], in1=tcv, op=Op.subtract)
                    nc.vector.tensor_scalar(out=xn_t[k], in0=x_t[k], scalar1=sc, scalar2=tcv,
                                            op0=Op.mult, op1=Op.add)
                # preload the exp table set right after the last sqrt so the
                # table switch is off the attention critical path
                dume = gn.tile([1, 1], f32, tag="dume", name="dume")
                nc.scalar.activation(out=dume, in_=eps_t[0:1, 0:1], func=Act.Exp, scale=1.0)

            # ---- QKV groups + software-pipelined attention + proj ----
            with (
                tc.tile_pool(name="att", bufs=3) as att,
                tc.tile_pool(name="pqkv", bufs=2, space="PSUM") as pqkv,
                tc.tile_pool(name="pS", bufs=2, space="PSUM") as pS,
                tc.tile_pool(name="pO", bufs=1, space="PSUM") as pO,
            ):
                gstate = {}

                def _half(key, lhs_of_k, rhs_of_k, h, fin):
                    # emit matmuls k in (0,1) for h==0 / (2,3) for h==1 of a
                    # 4-matmul accumulation group; fin() runs after the stop MM
                    if h == 0:
                        gstate[key] = pqkv.tile([128, 512], f32, tag="qkv", name="qkv")
                    ps = gstate[key]
                    for k in (0, 1) if h == 0 else (2, 3):
                        nc.tensor.matmul(ps, lhs_of_k(k), rhs_of_k(k),
                                         start=(k == 0), stop=(k == 3))
                    if h == 1:
                        fin(gstate.pop(key))

                def q_half(j, nt, h):
                    def fin(ps):
                        nc.vector.tensor_scalar_add(out=qh_t[nt][j], in0=ps,
                                                    scalar1=bqT_t[:, j:j + 1])
                    _half(("q", j, nt),
                          lambda k: wq_t[k][:, _qcol(j):_qcol(j) + 128],
                          lambda k: xn_t[k][:, nt * 512:(nt + 1) * 512], h, fin)

                def k_half(j, nt, h):
                    def fin(ps):
                        nc.vector.tensor_scalar_add(out=kf_t[j][:, nt * 512:(nt + 1) * 512],
                                                    in0=ps, scalar1=bqT_t[:, 4 + j:5 + j])
                    _half(("k", j, nt),
                          lambda k: wq_t[k][:, _kcol(j):_kcol(j) + 128],
                          lambda k: xn_t[k][:, nt * 512:(nt + 1) * 512], h, fin)

                def vt_half(mc, h):
                    def fin(ps):
                        nc.vector.tensor_tensor(out=vpf_t[:, mc, :, 0:HD],
                                                in0=ps.rearrange("p (h c) -> p h c", h=NH),
                                                in1=bvb_t.rearrange("p (h c) -> p h c", h=NH),
                                                op=Op.add)
                    _half(("v", mc),
                          lambda k: xn_t[k][:, mc * 128:(mc + 1) * 128],
                          lambda k: wq_t[k][:, VCOL:VCOL + 512], h, fin)

                def proj_half(oc, nt, h):
                    nsl = slice(nt * 512, (nt + 1) * 512)

                    def fin(ps):
                        nc.vector.scalar_tensor_tensor(out=y_t[oc][:, nsl], in0=ps,
                                                       scalar=bpT_t[:, oc:oc + 1],
                                                       in1=x_t[oc][:, nsl],
                                                       op0=Op.add, op1=Op.add)
                        nc.sync.dma_start(out=y_d[oc * 128:(oc + 1) * 128, nsl],
                                          in_=y_t[oc][:, nsl])
                    _half(("p", oc, nt),
                          lambda k: wp_t[k][:, oc * 128:(oc + 1) * 128],
                          lambda k: out_t[k][:, nsl], h, fin)

                def full(f, *a):
                    f(*a, 0)
                    f(*a, 1)

                # prologue groups: enough for attention iterations 0-1
                full(q_half, 0, 0)
                full(k_half, 0, 0)
                full(vt_half, 0)
                full(vt_half, 1)

                # just-in-time emission schedule (iteration index -> work).
                # q/k halves must finish emission strictly before the QK-pair
                # emission (at iteration I-1) that reads them.
                pending = {
                    0: [lambda: full(vt_half, 2)],
                    1: [lambda: full(vt_half, 3)],
                    2: [lambda: full(vt_half, 4), lambda: full(k_half, 0, 1)],
                    3: [lambda: full(vt_half, 5)],
                    4: [lambda: full(vt_half, 6)],
                    5: [lambda: full(vt_half, 7), lambda: full(k_half, 1, 0)],
                    6: [lambda: full(q_half, 1, 0)],
                    8: [lambda: k_half(1, 1, 0)], 9: [lambda: k_half(1, 1, 1)],
                    11: [lambda: k_half(2, 0, 0)], 12: [lambda: k_half(2, 0, 1)],
                    13: [lambda: q_half(2, 0, 0)], 14: [lambda: q_half(2, 0, 1)],
                    16: [lambda: k_half(2, 1, 0)], 17: [lambda: k_half(2, 1, 1)],
                    19: [lambda: k_half(3, 0, 0)], 20: [lambda: k_half(3, 0, 1)],
                    21: [lambda: q_half(3, 0, 0)], 22: [lambda: q_half(3, 0, 1)],
                    24: [lambda: k_half(3, 1, 0)], 25: [lambda: k_half(3, 1, 1)],
                    28: [lambda: q_half(0, 1, 0)], 29: [lambda: q_half(0, 1, 1)],
                    34: [lambda: q_half(1, 1, 0)], 35: [lambda: q_half(1, 1, 1)],
                    36: [lambda: proj_half(0, 0, 0)], 37: [lambda: proj_half(0, 0, 1)],
                    38: [lambda: proj_half(1, 0, 0)], 39: [lambda: proj_half(1, 0, 1)],
                    40: [lambda: proj_half(2, 0, 0)], 41: [lambda: proj_half(2, 0, 1)],
                    42: [lambda: proj_half(3, 0, 0)], 43: [lambda: proj_half(3, 0, 1)],
                    44: [lambda: q_half(2, 1, 0)], 45: [lambda: q_half(2, 1, 1)],
                    50: [lambda: q_half(3, 1, 0)], 51: [lambda: q_half(3, 1, 1)],
                }

                seq = [(nt, hp, mc) for nt in range(2) for hp in range(4) for mc in range(8)]

                def qk_pair(nt, hp, mc):
                    msl = slice(mc * 128, (mc + 1) * 128)
                    Sps = pS.tile([128, 1024], f32, tag="S", name="S")
                    nc.tensor.matmul(Sps[:, 0:512], kf_t[hp][0:64, msl], qh_t[nt][hp][0:64, :],
                                     start=True, stop=True)
                    nc.tensor.matmul(Sps[:, 512:1024], kf_t[hp][64:128, msl], qh_t[nt][hp][64:128, :],
                                     start=True, stop=True, tile_position=(64, 0))
                    return Sps

                # deferred per-head-pair normalization pieces; each is emitted a
                # little later so the Vector FIFO never convoys ahead of the
                # latency-critical qkv bias adds
                deferred = []

                def emit_norm(hp, nt, oAB):
                    nsl = slice(nt * 512, (nt + 1) * 512)
                    dq = nc.sync if nt == 1 else nc.gpsimd
                    rc2 = att.tile([1, 1024], f32, tag="rc2", name="rc2")
                    rc2b = att.tile([1, 1024], f32, tag="rc2b", name="rc2b")
                    bc2 = att.tile([64, 1024], f32, tag="bc2", name="bc2")
                    stagB = att.tile([64, 512], bf16, tag="stagB", name="stagB")

                    def p1():
                        dq.dma_start(out=rc2, in_=oAB[64:65, :])
                        nc.vector.reciprocal_approx_fast(out=rc2b[0:1, :], in_=rc2[0:1, :])

                    def p2():
                        nc.gpsimd.partition_broadcast(bc2, rc2b[0:1, :])
                        nc.vector.tensor_tensor(out=out_t[hp][0:64, nsl], in0=oAB[0:64, 0:512],
                                                in1=bc2[:, 0:512], op=Op.mult)

                    def p3():
                        nc.vector.tensor_tensor(out=stagB, in0=oAB[0:64, 512:1024],
                                                in1=bc2[:, 512:1024], op=Op.mult)
                        dq.dma_start(out=out_t[hp][64:128, nsl], in_=stagB)

                    deferred.extend([p1, p2, p3])

                cur_S = qk_pair(*seq[0])
                outAB = None
                for i, (nt, hp, mc) in enumerate(seq):
                    ex = att.tile([128, 1024], bf16, tag="ex", name="ex")
                    nc.scalar.activation(out=ex, in_=cur_S, func=Act.Exp, scale=0.125)
                    if i + 1 < len(seq):
                        cur_S = qk_pair(*seq[i + 1])
                    for g in pending.pop(i, []):
                        g()
                    if mc == 0:
                        outAB = pO.tile([HD + 1, 1024], f32, tag="outAB", name="outAB")
                    nc.tensor.matmul(outAB[:, 0:512], vpf_t[:, mc, 2 * hp, :], ex[:, 0:512],
                                     start=(mc == 0), stop=(mc == 7))
                    nc.tensor.matmul(outAB[:, 512:1024], vpf_t[:, mc, 2 * hp + 1, :], ex[:, 512:1024],
                                     start=(mc == 0), stop=(mc == 7))
                    if mc == 7:
                        # copy AV output to SBUF right away (frees PSUM for
                        # the next head-pair); the rest is deferred -- except
                        # for the last head-pair, whose chain runs inline
                        oAB = att.tile([65, 1024], f32, tag="oAB", name="oAB")
                        nc.vector.tensor_copy(out=oAB, in_=outAB)
                        emit_norm(hp, nt, oAB)
                        if i == len(seq) - 1:
                            while deferred:
                                deferred.pop(0)()
                    if deferred and i >= 2:
                        deferred.pop(0)()

                # tail: the nt=1 projections
                for oc in range(4):
                    proj_half(oc, 1, 0)
                    proj_half(oc, 1, 1)
                for k in range(4):
                    nc.gpsimd.dma_start(out=dbg_d[k * 128:(k + 1) * 128, :], in_=out_t[k])

    nc.compile()
    return nc


def _host_inputs(x, gamma, beta, w_qkv, b_qkv, w_proj, b_proj):
    import ml_dtypes
    f = np.float32
    bf = ml_dtypes.bfloat16
    # [B, C, N] -> per-core [128, 4, N] (partition-major chunk layout)
    xb = np.ascontiguousarray(np.asarray(x, f).reshape(NCORES, 4, 128, N)
                              .transpose(0, 2, 1, 3).astype(bf))
    wqkvT = np.asarray(w_qkv, f).T.reshape(4, 128, 3 * C).transpose(1, 0, 2)  # [128, 4, 3C]
    # column order: [q0 | k0 | v | k1 k2 k3 | q1 q2 q3]
    perm = np.concatenate([np.arange(0, 128), np.arange(512, 640), np.arange(1024, 1536),
                           np.arange(640, 1024), np.arange(128, 512)])
    wqkvT = np.ascontiguousarray(wqkvT[:, :, perm].astype(bf))
    wprojT = np.ascontiguousarray(np.asarray(w_proj, f).T.reshape(4, 128, C)
                                  .transpose(1, 0, 2).astype(bf))       # [128, 4, C]
    bq = np.asarray(b_qkv, f)
    cst = np.zeros((128, 20), f)
    cst[:, 0:4] = np.asarray(gamma, f).reshape(4, 128).T
    cst[:, 4:8] = np.asarray(beta, f).reshape(4, 128).T
    cst[:, 8:16] = bq[0:1024].reshape(8, 128).T
    cst[:, 16:20] = np.asarray(b_proj, f).reshape(4, 128).T
    bvb = np.ascontiguousarray(np.broadcast_to(bq[1024:1536][None, :], (128, 512)).astype(bf))
    gsel = np.zeros((128, 8), bf)
    bsel = np.zeros((8, 128), bf)
    for p in range(128):
        gsel[p, p // 16] = bf(1.0 / 16.0)
        bsel[p // 16, p] = bf(1.0)
    shared = {"wqkvT": wqkvT, "wprojT": wprojT, "cst": cst, "bvb": bvb,
              "gsel": gsel, "bsel": bsel}
    return [dict(shared, x=xb[i]) for i in range(NCORES)]


def run(inputs, trace=False, **kwargs):
    from concourse.bass_utils import run_bass_kernel_spmd
    if "nc" not in _CACHE:
        _CACHE["nc"] = _build_program()
    nc = _CACHE["nc"]
    in_maps = _host_inputs(**inputs)
    res = run_bass_kernel_spmd(nc, in_maps, core_ids=list(range(NCORES)), trace=trace, **kwargs)
    H = W = 32
    y = np.stack([np.asarray(res.results[i]["y"], dtype=np.float32).reshape(C, H, W) for i in range(NCORES)])
    return y.astype(np.float32), res


def kernel(**inputs):
    y, _ = run(inputs, trace=False)
    return y
